# revision 14
# baseline (speedup 1.0000x reference)
"""Curvphormer GNN on Trainium2, 8 NeuronCores, single fused Bass NEFF.

Design:
- Edges sharded by SRC across the 8 cores; node rows range-partitioned so each
  core's src range == its node shard (aggregation back to src is core-local).
- One NEFF runs the whole forward. Per layer:
    phase0: build node tables from all-gathered h:
            A_src[SL,260]=[h|q|hsq] (local shard), THK[NP,260]=[h|k|hsq],
            TV[NP,128]=v   (LayerNorm folded into q/k/v weights)
    phase1: tgt-sorted 128-edge tiles (aligned to 128-node windows):
            indirect-gather src/tgt rows, per-edge sim/dist/per-head qk;
            one-hot segment matmul accumulates [sum exp(sim), sum exp(sim)*dist]
            per tgt window into SBUF stats
    AllReduce stats1 [NP,2]
    phase1b: same tiles: broadcast agg=num/den back per edge (transposed
            one-hot matmul), curvature bias MLP (outer-product + matmul),
            e2=exp(qk/4+bias+ct) -> parray (DRAM) and den2 segment stats
    AllReduce stats2 [NP,8]; 1/den2 written to rden2d (DRAM, node rows)
    phase2: src-sorted tiles: indirect-gather v rows, permuted e2 rows, and
            rden2 rows; probs=e2*rden2; one-hot matmul accumulates messages
            into the local aggm (SBUF, src windows are core-local)
    dense:  h += aggm@oW+ob; h += FFN(LN2(h)) on local shard; AllGather h
- Segment softmax skips the per-segment max subtraction (shift-invariant;
  scores are O(1) at this model scale so exp cannot overflow in fp32).
- Mean-pool per graph + output MLP on device; y[64,1] read from core 0.

kernel() sorts/pads edges on host, compiles the SPMD program for the observed
tile counts, runs it via run_bass_kernel_spmd, and reports the steady-state
wall time of one full dispatch as _LAST_EXEC_NS.
"""

import sys
import time

import numpy as np

if "/opt/trn_rl_repo" not in sys.path:
    sys.path.insert(0, "/opt/trn_rl_repo")

P = 128
NC = 8
D = 128
H = 8
DH = 16
FI = 64
G = 64
L = 4
EPS = 1e-5

_LAST_EXEC_NS = 0


# ---------------------------------------------------------------------------
# host preprocessing
# ---------------------------------------------------------------------------

def _ceil_div(a, b):
    return (a + b - 1) // b


def _pack_side(order_key, aux0, aux1, win_base):
    """Sort the core's edges by order_key and pack into 128-edge tiles that
    never straddle a 128-id window (window = (key - win_base) // 128).
    Returns (rel, aux0, aux1, win, slot_of_edge) where slot_of_edge maps the
    pre-sort edge position to its tile slot (tile*128 + lane)."""
    ne = len(order_key)
    o = np.argsort(order_key, kind="stable")
    key, a0, a1 = order_key[o], aux0[o], aux1[o]
    rel_all = key - win_base
    wkey = rel_all // P
    tiles_rel, tiles_a0, tiles_a1, tiles_w = [], [], [], []
    slot_of_edge = np.zeros(ne, np.int64)
    if ne:
        bounds = np.flatnonzero(np.diff(wkey)) + 1
        starts = np.concatenate([[0], bounds])
        ends = np.concatenate([bounds, [ne]])
        for s, e in zip(starts, ends):
            w = int(wkey[s])
            for t in range(_ceil_div(e - s, P)):
                lo, hi = s + t * P, min(s + (t + 1) * P, e)
                m = hi - lo
                rel = np.full(P, -1.0, np.float32)
                b0 = np.zeros(P, np.int64)
                b1 = np.zeros(P, np.int64)
                rel[:m] = (rel_all[lo:hi] - w * P).astype(np.float32)
                b0[:m] = a0[lo:hi]
                b1[:m] = a1[lo:hi]
                ti = len(tiles_rel)
                slot_of_edge[o[lo:hi]] = ti * P + np.arange(m)
                tiles_rel.append(rel)
                tiles_a0.append(b0)
                tiles_a1.append(b1)
                tiles_w.append(w)
    return tiles_rel, tiles_a0, tiles_a1, tiles_w, slot_of_edge


def _host_prep(x, edge_index):
    n_nodes = x.shape[0]
    src = edge_index[0].astype(np.int64)
    tgt = edge_index[1].astype(np.int64)
    nwl = _ceil_div(_ceil_div(n_nodes, NC), P)
    SL = nwl * P
    NP_ = NC * SL
    NW = NC * nwl

    cores = []
    for c in range(NC):
        lo = c * SL
        mask = (src >= lo) & (src < lo + SL)
        es, et = src[mask], tgt[mask]
        ne = len(es)
        # phase1: tgt-sorted, global tgt windows
        rel1, a_src, a_tgt, w1, slot1 = _pack_side(et, es - lo, et, 0)
        # phase2: src-sorted, local windows; carry tgt and the phase1 slot
        rel2, b_tgt, b_slot, w2, _ = _pack_side(es, et, slot1, lo)
        t1, t2 = len(rel1), len(rel2)
        st = lambda a, dt: (np.stack(a).astype(dt) if a else
                            np.zeros((0, P), dt))
        cores.append(dict(
            t1=t1, t2=t2, ne=ne,
            rel1=st(rel1, np.float32), src1=st(a_src, np.int32),
            tgt1=st(a_tgt, np.int32), w1=np.array(w1, np.int64),
            rel2=st(rel2, np.float32), tgt2=st(b_tgt, np.int32),
            perm2=st(b_slot, np.int32), w2=np.array(w2, np.int64),
        ))
    meta = dict(nwl=nwl, SL=SL, NP=NP_, NW=NW,
                T1=max(1, max(cc["t1"] for cc in cores)),
                T2=max(1, max(cc["t2"] for cc in cores)))
    return meta, cores


def _pad_tiles(arr, T, fill, dtype):
    out = np.full((T, P), fill, dtype)
    if arr.shape[0]:
        out[: arr.shape[0]] = arr
    return out


def _core_inputs(meta, core, x, batch, n_nodes, c):
    SL, NP_, NW, T1, T2 = (meta[k] for k in ("SL", "NP", "NW", "T1", "T2"))
    clo = c * SL

    rel1 = _pad_tiles(core["rel1"], T1, -1.0, np.float32)
    src1 = _pad_tiles(core["src1"], T1, 0, np.int32)
    tgt1 = _pad_tiles(core["tgt1"], T1, 0, np.int32)
    rel2 = _pad_tiles(core["rel2"], T2, -1.0, np.float32)
    tgt2 = _pad_tiles(core["tgt2"], T2, 0, np.int32)
    perm2 = _pad_tiles(core["perm2"], T2, 0, np.int32)
    # rden2d row index for node n: (n % P) * NW + n // P
    tgtd2 = ((tgt2 % P) * NW + tgt2 // P).astype(np.int32)

    w1 = np.zeros(T1, np.int64)
    w1[: core["t1"]] = core["w1"]
    w2 = np.zeros(T2, np.int64)
    w2[: core["t2"]] = core["w2"]
    offs1 = np.stack([w1 * 2, w1 * 8, w1], axis=1).astype(np.int32)
    offs2 = (w2 * P).astype(np.int32)

    xs = np.zeros((SL, FI), np.float32)
    hi = min(clo + SL, n_nodes)
    if hi > clo:
        xs[: hi - clo] = x[clo:hi]

    batchrel = np.full(NP_, -1.0, np.float32)
    batchrel[:n_nodes] = batch.astype(np.float32)
    batchrel = np.ascontiguousarray(batchrel.reshape(NW, P).T)

    return dict(
        x=xs,
        p1_rel=np.ascontiguousarray(rel1.T),
        p1_src=np.ascontiguousarray(src1.T),
        p1_tgt=np.ascontiguousarray(tgt1.T),
        p1_offs=np.ascontiguousarray(offs1.reshape(1, -1)),
        p2_rel=np.ascontiguousarray(rel2.T),
        p2_tgt=np.ascontiguousarray(tgt2.T),
        p2_tgtd=np.ascontiguousarray(tgtd2.T),
        p2_perm=np.ascontiguousarray(perm2.T),
        p2_offs=np.ascontiguousarray(offs2.reshape(1, -1)),
        batchrel=batchrel,
    )


def _fold_weights(w):
    out = {}
    out["nodeW"] = w["node_W"].astype(np.float32)
    out["nodeb"] = w["node_b"].astype(np.float32).reshape(1, D)
    out["cw1"] = w["cW1"].astype(np.float32).reshape(1, D)      # row
    out["cb1"] = w["cb1"].astype(np.float32).reshape(D, 1)      # col
    qs = 1.0 / np.sqrt(DH)
    for l in range(L):
        n1s, n1b = w["n1s"][l], w["n1b"][l]
        qW = n1s[:, None] * w["qW"][l]
        kW = n1s[:, None] * w["kW"][l]
        vW = n1s[:, None] * w["vW"][l]
        qb = w["qb"][l] + n1b @ w["qW"][l]
        kb = w["kb"][l] + n1b @ w["kW"][l]
        vb = w["vb"][l] + n1b @ w["vW"][l]
        out[f"Wqkv{l}"] = np.concatenate(
            [qW * qs, kW, vW], axis=1).astype(np.float32)
        out[f"bqkv{l}"] = np.concatenate(
            [qb * qs, kb, vb]).astype(np.float32).reshape(1, 3 * D)
        out[f"oW{l}"] = w["oW"][l].astype(np.float32)
        out[f"ob{l}"] = w["ob"][l].astype(np.float32).reshape(1, D)
        n2s, n2b = w["n2s"][l], w["n2b"][l]
        out[f"f1W{l}"] = (n2s[:, None] * w["f1W"][l]).astype(np.float32)
        out[f"f1b{l}"] = (w["f1b"][l] + n2b @ w["f1W"][l]
                          ).astype(np.float32).reshape(1, 4 * D)
        f2 = w["f2W"][l].astype(np.float32).reshape(4, P, D)
        out[f"f2W{l}"] = np.ascontiguousarray(
            f2.transpose(1, 0, 2)).reshape(P, 4 * D)
        out[f"f2b{l}"] = w["f2b"][l].astype(np.float32).reshape(1, D)
        out[f"Bt{l}"] = (w["cW2"] @ w["bW"][l]).astype(np.float32)
        out[f"ct{l}"] = (w["cb2"] @ w["bW"][l] + w["bb"][l]
                         ).astype(np.float32).reshape(H, 1)
    out["outW1"] = w["outW1"].astype(np.float32)
    out["outb1"] = w["outb1"].astype(np.float32).reshape(1, G)
    out["outW2"] = w["outW2"].astype(np.float32)
    out["outb2"] = w["outb2"].astype(np.float32).reshape(1, 1)
    return out


# ---------------------------------------------------------------------------
# device program
# ---------------------------------------------------------------------------

def _build_program(meta):
    import concourse.bass as bass
    import concourse.tile as tile
    import concourse.mybir as mybir
    import concourse.bacc as bacc
    from concourse.masks import make_identity

    f32 = mybir.dt.float32
    i32 = mybir.dt.int32
    Alu = mybir.AluOpType
    Act = mybir.ActivationFunctionType
    Ax = mybir.AxisListType
    IOff = bass.IndirectOffsetOnAxis

    SL, NP_, NW, NWL, T1, T2 = (
        meta[k] for k in ("SL", "NP", "NW", "nwl", "T1", "T2"))
    import os
    STAGE = int(os.environ.get("KDEV_STAGE", "99"))
    NLAYERS = int(os.environ.get("KDEV_LAYERS", str(L)))

    nc = bacc.Bacc(None, target_bir_lowering=False, num_devices=NC)
    RG = [[i for i in range(NC)]]

    def din(name, shape, dt=f32):
        return nc.declare_dram_parameter(name, list(shape), dt, isOutput=False)

    x_in = din("x", [SL, FI])
    p1_rel = din("p1_rel", [P, T1])
    p1_src = din("p1_src", [P, T1], i32)
    p1_tgt = din("p1_tgt", [P, T1], i32)
    p1_offs = din("p1_offs", [1, T1 * 3], i32)
    p2_rel = din("p2_rel", [P, T2])
    p2_tgt = din("p2_tgt", [P, T2], i32)
    p2_tgtd = din("p2_tgtd", [P, T2], i32)
    p2_perm = din("p2_perm", [P, T2], i32)
    p2_offs = din("p2_offs", [1, T2], i32)
    batchrel = din("batchrel", [P, NW])
    ginv_in = din("ginv", [G, 1])

    wshapes = dict(
        nodeW=[FI, D], nodeb=[1, D], cw1=[1, D], cb1=[D, 1],
        outW1=[D, G], outb1=[1, G], outW2=[G, 1], outb2=[1, 1])
    for l in range(L):
        wshapes.update({
            f"Wqkv{l}": [D, 3 * D], f"bqkv{l}": [1, 3 * D],
            f"oW{l}": [D, D], f"ob{l}": [1, D],
            f"f1W{l}": [D, 4 * D], f"f1b{l}": [1, 4 * D],
            f"f2W{l}": [P, 4 * D], f"f2b{l}": [1, D],
            f"Bt{l}": [D, H], f"ct{l}": [H, 1]})
    wt = {n: din("w_" + n, s) for n, s in wshapes.items()}

    y_out = nc.declare_dram_parameter("y", [G, 1], f32, isOutput=True)
    DUMP = int(os.environ.get("KDEV_DUMP", "0"))
    dbg = {}
    if DUMP:
        for nm, shp in (("dbg_h0", [NP_, D]), ("dbg_thk", [NP_, 260]),
                        ("dbg_asrc", [SL, 260]), ("dbg_st1", [P, NW * 2]),
                        ("dbg_aggd", [P, NW]), ("dbg_st2", [P, NW * 8]),
                        ("dbg_parray", [T1 * P, H]),
                        ("dbg_aggm", [P, NWL * P]), ("dbg_h1", [NP_, D])):
            dbg[nm] = nc.declare_dram_parameter(nm, shp, f32, isOutput=True)

    def dram(name, shape, shared=False):
        return nc.dram_tensor(name, list(shape), f32, kind="Internal",
                              addr_space="Shared" if shared else "Local")

    ag_in = [dram(f"agin{l}", [SL, D]) for l in range(L + 1)]
    h_full = [dram(f"hfull{l}", [NP_, D], shared=True) for l in range(L + 1)]
    st1_in = [dram(f"st1i{l}", [P, NW * 2]) for l in range(L)]
    st1_out = [dram(f"st1o{l}", [P, NW * 2], shared=True) for l in range(L)]
    st2_in = [dram(f"st2i{l}", [P, NW * 8]) for l in range(L)]
    st2_out = [dram(f"st2o{l}", [P, NW * 8], shared=True) for l in range(L)]
    A_src = dram("A_src", [SL, 260])
    THK = dram("THK", [NP_, 260])
    TV = dram("TV", [NP_, D])
    parray = dram("parray", [T1 * P, H])
    rden2d = dram("rden2d", [P * NW, 8])

    from contextlib import ExitStack
    with tile.TileContext(nc) as tc, ExitStack() as stack:
        cpool = stack.enter_context(tc.tile_pool(name="cpool", bufs=1))
        mpool = stack.enter_context(tc.tile_pool(name="meta", bufs=1))
        spool = stack.enter_context(tc.tile_pool(name="state", bufs=1))
        wpool = stack.enter_context(tc.tile_pool(name="weights", bufs=1))
        pool = stack.enter_context(tc.tile_pool(name="work", bufs=2))
        ppool = stack.enter_context(
            tc.tile_pool(name="psum", bufs=6, space="PSUM"))
        gpool = stack.enter_context(
            tc.tile_pool(name="psumg", bufs=1, space="PSUM"))

        ident = cpool.tile([P, P], f32, tag="ident")
        make_identity(nc, ident[:])
        ones_row = cpool.tile([1, P], f32, tag="ones")
        nc.vector.memset(ones_row[:], 1.0)
        iota_row_i = cpool.tile([P, P], i32, tag="iotai")
        nc.gpsimd.iota(iota_row_i[:], pattern=[[1, P]], channel_multiplier=0)
        iota_row = cpool.tile([P, P], f32, tag="iotaf")
        nc.vector.tensor_copy(iota_row[:], iota_row_i[:])
        iota64_i = cpool.tile([P, G], i32, tag="iota64i")
        nc.gpsimd.iota(iota64_i[:], pattern=[[1, G]], channel_multiplier=0)
        iota64 = cpool.tile([P, G], f32, tag="iota64f")
        nc.vector.tensor_copy(iota64[:], iota64_i[:])

        m_rel1 = mpool.tile([P, T1], f32, tag="rel1")
        m_src1 = mpool.tile([P, T1], i32, tag="src1")
        m_tgt1 = mpool.tile([P, T1], i32, tag="tgt1")
        m_offs1 = mpool.tile([1, T1 * 3], i32, tag="offs1")
        m_rel2 = mpool.tile([P, T2], f32, tag="rel2")
        m_tgt2 = mpool.tile([P, T2], i32, tag="tgt2")
        m_tgtd2 = mpool.tile([P, T2], i32, tag="tgtd2")
        m_perm2 = mpool.tile([P, T2], i32, tag="perm2")
        m_offs2 = mpool.tile([1, T2], i32, tag="offs2")
        m_brel = mpool.tile([P, NW], f32, tag="brel")
        for t_, s_ in ((m_rel1, p1_rel), (m_src1, p1_src), (m_tgt1, p1_tgt),
                       (m_offs1, p1_offs), (m_rel2, p2_rel), (m_tgt2, p2_tgt),
                       (m_tgtd2, p2_tgtd), (m_perm2, p2_perm),
                       (m_offs2, p2_offs), (m_brel, batchrel)):
            nc.sync.dma_start(out=t_[:], in_=s_[:, :])
        cw1_row = cpool.tile([1, D], f32, tag="cw1")
        cb1_col = cpool.tile([D, 1], f32, tag="cb1")
        nc.sync.dma_start(out=cw1_row[:], in_=wt["cw1"][:, :])
        nc.sync.dma_start(out=cb1_col[:], in_=wt["cb1"][:, :])
        ginv = cpool.tile([G, 1], f32, tag="ginv")
        nc.sync.dma_start(out=ginv[:], in_=ginv_in[:, :])

        estore = spool.tile([P, T1 * 9], f32, tag="estore")
        stats1 = spool.tile([P, NW * 2], f32, tag="stats1")
        stats2 = spool.tile([P, NW * 8], f32, tag="stats2")
        aggd = spool.tile([P, NW], f32, tag="aggd")
        rden2 = spool.tile([P, NW * 8], f32, tag="rden2")
        aggm = spool.tile([P, NWL * P], f32, tag="aggm")

        def ln_stats(h_ap, hsqs_out, scratch_pref):
            """LayerNorm helpers: writes sum-of-squares to hsqs_out, returns
            (mu, rstd) pool tiles."""
            tmp = pool.tile([P, D], f32, tag=scratch_pref + "t")
            nc.scalar.activation(tmp[:], h_ap, Act.Square,
                                 accum_out=hsqs_out)
            mus = pool.tile([P, 1], f32, tag=scratch_pref + "m")
            nc.vector.tensor_reduce(out=mus[:], in_=h_ap, axis=Ax.X,
                                    op=Alu.add)
            var = pool.tile([P, 4], f32, tag=scratch_pref + "v")
            nc.vector.tensor_tensor(out=var[:, 0:1], in0=mus[:], in1=mus[:],
                                    op=Alu.mult)
            nc.vector.tensor_scalar(out=var[:, 1:2], in0=hsqs_out,
                                    scalar1=float(P), scalar2=None,
                                    op0=Alu.mult)
            nc.vector.tensor_tensor(out=var[:, 2:3], in0=var[:, 1:2],
                                    in1=var[:, 0:1], op=Alu.subtract)
            nc.vector.tensor_scalar(out=var[:, 3:4], in0=var[:, 2:3],
                                    scalar1=1.0 / (P * P), scalar2=EPS,
                                    op0=Alu.mult, op1=Alu.add)
            std = pool.tile([P, 1], f32, tag=scratch_pref + "s")
            nc.scalar.sqrt(std[:], var[:, 3:4])
            rstd = pool.tile([P, 1], f32, tag=scratch_pref + "r")
            nc.vector.reciprocal(rstd[:], std[:])
            mu = pool.tile([P, 1], f32, tag=scratch_pref + "u")
            nc.vector.tensor_scalar(out=mu[:], in0=mus[:], scalar1=1.0 / P,
                                    scalar2=None, op0=Alu.mult)
            return mu, rstd

        # ---- initial projection ----
        w_nodeW = wpool.tile([FI, D], f32, tag="w0")
        w_nodeb = wpool.tile([1, D], f32, tag="w0b")
        nc.sync.dma_start(out=w_nodeW[:], in_=wt["nodeW"][:, :])
        nc.sync.dma_start(out=w_nodeb[:], in_=wt["nodeb"][:, :])
        for lw in range(NWL):
            xt = pool.tile([P, FI], f32, tag="xt")
            nc.sync.dma_start(out=xt[:], in_=x_in[lw * P:(lw + 1) * P, :])
            xT_ps = ppool.tile([P, P], f32, tag="ps")
            nc.tensor.transpose(out=xT_ps[:FI, :], in_=xt[:],
                                identity=ident[:])
            xT = pool.tile([FI, P], f32, tag="xT")
            nc.vector.tensor_copy(xT[:], xT_ps[:FI, :])
            h_ps = ppool.tile([P, D], f32, tag="ps")
            nc.tensor.matmul(out=h_ps[:], lhsT=xT[:], rhs=w_nodeW[:],
                             start=True, stop=False)
            nc.tensor.matmul(out=h_ps[:], lhsT=ones_row[:], rhs=w_nodeb[:],
                             start=False, stop=True)
            h_sb = pool.tile([P, D], f32, tag="hsb")
            nc.vector.tensor_copy(h_sb[:], h_ps[:])
            nc.sync.dma_start(out=ag_in[0][lw * P:(lw + 1) * P, :],
                              in_=h_sb[:])
        nc.gpsimd.collective_compute(
            "AllGather", Alu.bypass, ins=[ag_in[0][:, :]],
            outs=[h_full[0][:, :]], replica_groups=RG)

        if DUMP:
            nc.sync.dma_start(out=dbg["dbg_h0"][:, :], in_=h_full[0][:, :])
        rank_sp = nc.partition_id(engines=[mybir.EngineType.SP])

        for l in range(min(L, NLAYERS)):
            hf = h_full[l]
            w_qkv = wpool.tile([D, 3 * D], f32, tag="wqkv")
            b_qkv = wpool.tile([1, 3 * D], f32, tag="bqkv")
            nc.sync.dma_start(out=w_qkv[:], in_=wt[f"Wqkv{l}"][:, :])
            nc.sync.dma_start(out=b_qkv[:], in_=wt[f"bqkv{l}"][:, :])

            def node_window(h_sb, dst260, mid_slice):
                """dst260[:,0:128]=h, [:,128:256]=qkv[mid_slice],
                [:,256]=sum(h^2); returns qkv psum tile."""
                mu, rstd = ln_stats(h_sb[:], dst260[:, 256:257], "ln")
                z = pool.tile([P, D], f32, tag="z")
                nc.vector.tensor_scalar(out=z[:], in0=h_sb[:],
                                        scalar1=mu[:], scalar2=rstd[:],
                                        op0=Alu.subtract, op1=Alu.mult)
                zT_ps = ppool.tile([P, P], f32, tag="ps")
                nc.tensor.transpose(out=zT_ps[:], in_=z[:],
                                    identity=ident[:])
                zT = pool.tile([P, P], f32, tag="zT")
                nc.vector.tensor_copy(zT[:], zT_ps[:])
                qkv_ps = ppool.tile([P, 3 * D], f32, tag="ps")
                nc.tensor.matmul(out=qkv_ps[:], lhsT=zT[:], rhs=w_qkv[:],
                                 start=True, stop=False)
                nc.tensor.matmul(out=qkv_ps[:], lhsT=ones_row[:],
                                 rhs=b_qkv[:], start=False, stop=True)
                nc.vector.tensor_copy(dst260[:, 0:128], h_sb[:])
                nc.vector.tensor_copy(dst260[:, 128:256],
                                      qkv_ps[:, mid_slice])
                return qkv_ps

            for w in range(NW if STAGE >= 1 else 0):
                h_sb = pool.tile([P, D], f32, tag="hsb")
                nc.sync.dma_start(out=h_sb[:], in_=hf[w * P:(w + 1) * P, :])
                thk = pool.tile([P, 260], f32, tag="thk")
                qkv_ps = node_window(h_sb, thk, slice(128, 256))
                v_sb = pool.tile([P, D], f32, tag="vsb")
                nc.vector.tensor_copy(v_sb[:], qkv_ps[:, 256:384])
                nc.sync.dma_start(out=THK[w * P:(w + 1) * P, :], in_=thk[:])
                nc.sync.dma_start(out=TV[w * P:(w + 1) * P, :], in_=v_sb[:])
            for lw in range(NWL if STAGE >= 2 else 0):
                h_sb = pool.tile([P, D], f32, tag="hsb")
                nc.sync.dma_start(
                    out=h_sb[:], in_=hf[bass.ds(rank_sp * SL + lw * P, P), :])
                asr = pool.tile([P, 260], f32, tag="asr")
                node_window(h_sb, asr, slice(0, 128))
                nc.sync.dma_start(out=A_src[lw * P:(lw + 1) * P, :],
                                  in_=asr[:])

            if DUMP and l == 0:
                nc.sync.dma_start(out=dbg["dbg_thk"][:, :], in_=THK[:, :])
                nc.sync.dma_start(out=dbg["dbg_asrc"][:, :], in_=A_src[:, :])
            # ---- phase 1 ----
            nc.vector.memset(stats1[:], 0.0)
            for t in range(T1 if STAGE >= 3 else 0):
                hs = pool.tile([P, 260], f32, tag="ghs")
                nc.gpsimd.indirect_dma_start(
                    out=hs[:], out_offset=None, in_=A_src[:, :],
                    in_offset=IOff(ap=m_src1[:, t:t + 1], axis=0))
                ht = pool.tile([P, 260], f32, tag="ght")
                nc.gpsimd.indirect_dma_start(
                    out=ht[:], out_offset=None, in_=THK[:, :],
                    in_offset=IOff(ap=m_tgt1[:, t:t + 1], axis=0))
                est = estore[:, t * 9:(t + 1) * 9]
                scr = pool.tile([P, D], f32, tag="scr")
                sim = pool.tile([P, 1], f32, tag="sim")
                nc.vector.tensor_tensor(out=scr[:], in0=hs[:, 0:128],
                                        in1=ht[:, 0:128], op=Alu.mult)
                nc.vector.tensor_reduce(out=sim[:], in_=scr[:], axis=Ax.X,
                                        op=Alu.add)
                qk = pool.tile([P, D], f32, tag="qk")
                nc.vector.tensor_tensor(out=qk[:], in0=hs[:, 128:256],
                                        in1=ht[:, 128:256], op=Alu.mult)
                nc.vector.tensor_reduce(
                    out=est[:, 0:8],
                    in_=qk[:].rearrange("p (h d) -> p h d", h=H),
                    axis=Ax.X, op=Alu.add)
                hsq2 = pool.tile([P, 1], f32, tag="hsq2")
                nc.vector.tensor_tensor(out=hsq2[:], in0=hs[:, 256:257],
                                        in1=ht[:, 256:257], op=Alu.add)
                d2 = pool.tile([P, 2], f32, tag="d2")
                nc.vector.tensor_scalar(out=d2[:, 0:1], in0=sim[:],
                                        scalar1=-2.0, scalar2=hsq2[:],
                                        op0=Alu.mult, op1=Alu.add)
                nc.vector.tensor_scalar(out=d2[:, 1:2], in0=d2[:, 0:1],
                                        scalar1=0.0, scalar2=None,
                                        op0=Alu.max)
                nc.scalar.sqrt(est[:, 8:9], d2[:, 1:2])
                nc.vector.tensor_scalar(out=sim[:], in0=sim[:],
                                        scalar1=-60.0, scalar2=60.0,
                                        op0=Alu.max, op1=Alu.min)
                vals = pool.tile([P, 2], f32, tag="vals")
                nc.scalar.activation(vals[:, 0:1], sim[:], Act.Exp)
                nc.vector.tensor_tensor(out=vals[:, 1:2], in0=vals[:, 0:1],
                                        in1=est[:, 8:9], op=Alu.mult)
                M = pool.tile([P, P], f32, tag="M")
                nc.vector.tensor_tensor(
                    out=M[:], in0=m_rel1[:, t:t + 1].to_broadcast([P, P]),
                    in1=iota_row[:], op=Alu.is_equal)
                s_ps = ppool.tile([P, 2], f32, tag="ps")
                nc.tensor.matmul(out=s_ps[:], lhsT=M[:], rhs=vals[:],
                                 start=True, stop=True)
                r_off = nc.alloc_register(mybir.EngineType.DVE)
                nc.vector.reg_load(r_off, m_offs1[0:1, 3 * t:3 * t + 1])
                off2 = nc.snap(r_off, donate=True, min_val=0,
                               max_val=max(0, (NW - 1) * 2))
                sl = stats1[:, bass.ds(off2, 2)]
                nc.vector.tensor_tensor(out=sl, in0=sl, in1=s_ps[:],
                                        op=Alu.add)

            if STAGE >= 4:
                nc.sync.dma_start(out=st1_in[l][:, :], in_=stats1[:])
                nc.gpsimd.collective_compute(
                    "AllReduce", Alu.add, ins=[st1_in[l][:, :]],
                    outs=[st1_out[l][:, :]], replica_groups=RG)
                nc.sync.dma_start(out=stats1[:], in_=st1_out[l][:, :])
            den_s = stats1[:].rearrange("p (w c) -> p w c", c=2)
            tmpd = pool.tile([P, NW], f32, tag="tmpd")
            nc.vector.tensor_scalar(out=tmpd[:], in0=den_s[:, :, 0],
                                    scalar1=1e-30, scalar2=None, op0=Alu.max)
            rden1 = pool.tile([P, NW], f32, tag="rden1")
            nc.vector.reciprocal(rden1[:], tmpd[:])
            nc.vector.tensor_tensor(out=aggd[:], in0=den_s[:, :, 1],
                                    in1=rden1[:], op=Alu.mult)

            if DUMP and l == 0:
                nc.sync.dma_start(out=dbg["dbg_st1"][:, :], in_=stats1[:])
                nc.sync.dma_start(out=dbg["dbg_aggd"][:, :], in_=aggd[:])
            # ---- phase 1b ----
            nc.vector.memset(stats2[:], 0.0)
            w_Bt = wpool.tile([D, H], f32, tag="wBt")
            T1b = T1 if STAGE >= 5 else 0
            nc.sync.dma_start(out=w_Bt[:], in_=wt[f"Bt{l}"][:, :])
            w_ct = wpool.tile([H, 1], f32, tag="wct")
            nc.sync.dma_start(out=w_ct[:], in_=wt[f"ct{l}"][:, :])
            for t in range(T1b):
                est = estore[:, t * 9:(t + 1) * 9]
                pkq_ps = ppool.tile([P, P], f32, tag="ps")
                nc.tensor.transpose(out=pkq_ps[:8, :], in_=est[:, 0:8],
                                    identity=ident[:])
                pkq = pool.tile([8, P], f32, tag="pkq")
                nc.vector.tensor_copy(pkq[:], pkq_ps[:8, :])
                pkd_ps = ppool.tile([P, P], f32, tag="ps")
                nc.tensor.transpose(out=pkd_ps[:1, :], in_=est[:, 8:9],
                                    identity=ident[:])
                pkd = pool.tile([1, P], f32, tag="pkd")
                nc.vector.tensor_copy(pkd[:], pkd_ps[:1, :])
                M = pool.tile([P, P], f32, tag="M")
                nc.vector.tensor_tensor(
                    out=M[:], in0=m_rel1[:, t:t + 1].to_broadcast([P, P]),
                    in1=iota_row[:], op=Alu.is_equal)
                MT_ps = ppool.tile([P, P], f32, tag="ps")
                nc.tensor.transpose(out=MT_ps[:], in_=M[:],
                                    identity=ident[:])
                MT = pool.tile([P, P], f32, tag="MT")
                nc.vector.tensor_copy(MT[:], MT_ps[:])
                r1 = nc.alloc_register(mybir.EngineType.DVE)
                nc.vector.reg_load(r1, m_offs1[0:1, 3 * t + 2:3 * t + 3])
                off1 = nc.snap(r1, donate=True, min_val=0,
                               max_val=max(0, NW - 1))
                stg = pool.tile([P, 1], f32, tag="stg")
                nc.vector.tensor_copy(stg[:], aggd[:, bass.ds(off1, 1)])
                agg_ps = ppool.tile([1, P], f32, tag="ps")
                nc.tensor.matmul(out=agg_ps[:], lhsT=stg[:], rhs=MT[:],
                                 start=True, stop=True)
                rd = pool.tile([1, P], f32, tag="rd")
                nc.vector.tensor_scalar(out=rd[:], in0=pkd[:],
                                        scalar1=1e-6, scalar2=None,
                                        op0=Alu.max)
                rdist = pool.tile([1, P], f32, tag="rdist")
                nc.vector.reciprocal(rdist[:], rd[:])
                curv = pool.tile([1, P], f32, tag="curv")
                nc.vector.tensor_tensor(out=curv[:], in0=agg_ps[:],
                                        in1=rdist[:], op=Alu.mult)
                nc.vector.tensor_scalar(out=curv[:], in0=curv[:],
                                        scalar1=-1.0, scalar2=1.0,
                                        op0=Alu.mult, op1=Alu.add)
                outer_ps = ppool.tile([P, P], f32, tag="ps")
                nc.tensor.matmul(out=outer_ps[:], lhsT=cw1_row[:],
                                 rhs=curv[:], start=True, stop=True)
                rT = pool.tile([P, P], f32, tag="rT")
                nc.vector.tensor_scalar(out=rT[:], in0=outer_ps[:],
                                        scalar1=cb1_col[:], scalar2=0.0,
                                        op0=Alu.add, op1=Alu.max)
                bias_ps = ppool.tile([H, P], f32, tag="ps")
                nc.tensor.matmul(out=bias_ps[:], lhsT=w_Bt[:], rhs=rT[:],
                                 start=True, stop=True)
                sc = pool.tile([H, P], f32, tag="sc")
                nc.vector.tensor_tensor(out=sc[:], in0=pkq[:],
                                        in1=bias_ps[:], op=Alu.add)
                scc = pool.tile([H, P], f32, tag="scc")
                nc.vector.tensor_scalar(out=scc[:], in0=sc[:],
                                        scalar1=w_ct[:], scalar2=None,
                                        op0=Alu.add)
                nc.vector.tensor_scalar(out=scc[:], in0=scc[:],
                                        scalar1=-60.0, scalar2=60.0,
                                        op0=Alu.max, op1=Alu.min)
                e2 = pool.tile([H, P], f32, tag="e2")
                nc.scalar.activation(e2[:], scc[:], Act.Exp)
                e2_ps = ppool.tile([P, H], f32, tag="ps")
                nc.tensor.transpose(out=e2_ps[:, :], in_=e2[:],
                                    identity=ident[0:H, 0:H])
                e2c = pool.tile([P, H], f32, tag="e2c")
                nc.vector.tensor_copy(e2c[:], e2_ps[:])
                nc.sync.dma_start(out=parray[t * P:(t + 1) * P, :],
                                  in_=e2c[:])
                d2_ps = ppool.tile([P, H], f32, tag="ps")
                nc.tensor.matmul(out=d2_ps[:], lhsT=M[:], rhs=e2c[:],
                                 start=True, stop=True)
                r2 = nc.alloc_register(mybir.EngineType.DVE)
                nc.vector.reg_load(r2, m_offs1[0:1, 3 * t + 1:3 * t + 2])
                off8 = nc.snap(r2, donate=True, min_val=0,
                               max_val=max(0, (NW - 1) * 8))
                sl2 = stats2[:, bass.ds(off8, 8)]
                nc.vector.tensor_tensor(out=sl2, in0=sl2, in1=d2_ps[:],
                                        op=Alu.add)

            if STAGE >= 6:
                nc.sync.dma_start(out=st2_in[l][:, :], in_=stats2[:])
                nc.gpsimd.collective_compute(
                    "AllReduce", Alu.add, ins=[st2_in[l][:, :]],
                    outs=[st2_out[l][:, :]], replica_groups=RG)
                nc.sync.dma_start(out=stats2[:], in_=st2_out[l][:, :])
            nc.vector.tensor_scalar(out=stats2[:], in0=stats2[:],
                                    scalar1=1e-32, scalar2=None, op0=Alu.max)
            nc.vector.reciprocal(rden2[:], stats2[:])
            nc.sync.dma_start(
                out=rden2d[:, :].rearrange("(p w) c -> p (w c)", p=P),
                in_=rden2[:])

            if DUMP and l == 0:
                nc.sync.dma_start(out=dbg["dbg_st2"][:, :], in_=stats2[:])
                nc.sync.dma_start(out=dbg["dbg_parray"][:, :],
                                  in_=parray[:, :])
            # ---- phase 2 ----
            nc.vector.memset(aggm[:], 0.0)
            for t in range(T2 if STAGE >= 7 else 0):
                vt = pool.tile([P, D], f32, tag="gv")
                nc.gpsimd.indirect_dma_start(
                    out=vt[:], out_offset=None, in_=TV[:, :],
                    in_offset=IOff(ap=m_tgt2[:, t:t + 1], axis=0))
                et_ = pool.tile([P, H], f32, tag="ge")
                nc.gpsimd.indirect_dma_start(
                    out=et_[:], out_offset=None, in_=parray[:, :],
                    in_offset=IOff(ap=m_perm2[:, t:t + 1], axis=0))
                rdt = pool.tile([P, H], f32, tag="gr")
                nc.gpsimd.indirect_dma_start(
                    out=rdt[:], out_offset=None, in_=rden2d[:, :],
                    in_offset=IOff(ap=m_tgtd2[:, t:t + 1], axis=0))
                probs = pool.tile([P, H], f32, tag="probs")
                nc.vector.tensor_tensor(out=probs[:], in0=et_[:], in1=rdt[:],
                                        op=Alu.mult)
                msgs = pool.tile([P, D], f32, tag="msgs")
                nc.vector.tensor_tensor(
                    out=msgs[:].rearrange("p (h d) -> p h d", h=H),
                    in0=vt[:].rearrange("p (h d) -> p h d", h=H),
                    in1=probs[:].rearrange("p (h o) -> p h o",
                                           o=1).to_broadcast([P, H, DH]),
                    op=Alu.mult)
                Ms = pool.tile([P, P], f32, tag="Ms")
                nc.vector.tensor_tensor(
                    out=Ms[:], in0=m_rel2[:, t:t + 1].to_broadcast([P, P]),
                    in1=iota_row[:], op=Alu.is_equal)
                a_ps = ppool.tile([P, D], f32, tag="ps")
                nc.tensor.matmul(out=a_ps[:], lhsT=Ms[:], rhs=msgs[:],
                                 start=True, stop=True)
                r3 = nc.alloc_register(mybir.EngineType.DVE)
                nc.vector.reg_load(r3, m_offs2[0:1, t:t + 1])
                offa = nc.snap(r3, donate=True, min_val=0,
                               max_val=max(0, (NWL - 1) * P))
                sla = aggm[:, bass.ds(offa, P)]
                nc.vector.tensor_tensor(out=sla, in0=sla, in1=a_ps[:],
                                        op=Alu.add)

            if DUMP and l == 0:
                nc.sync.dma_start(out=dbg["dbg_aggm"][:, :], in_=aggm[:])
            # ---- dense update ----
            w_o = wpool.tile([D, D], f32, tag="wo")
            b_o = wpool.tile([1, D], f32, tag="bo")
            w_f1 = wpool.tile([D, 4 * D], f32, tag="wf1")
            b_f1 = wpool.tile([1, 4 * D], f32, tag="bf1")
            w_f2 = wpool.tile([P, 4 * D], f32, tag="wf2")
            b_f2 = wpool.tile([1, D], f32, tag="bf2")
            nc.sync.dma_start(out=w_o[:], in_=wt[f"oW{l}"][:, :])
            nc.sync.dma_start(out=b_o[:], in_=wt[f"ob{l}"][:, :])
            nc.sync.dma_start(out=w_f1[:], in_=wt[f"f1W{l}"][:, :])
            nc.sync.dma_start(out=b_f1[:], in_=wt[f"f1b{l}"][:, :])
            nc.sync.dma_start(out=w_f2[:], in_=wt[f"f2W{l}"][:, :])
            nc.sync.dma_start(out=b_f2[:], in_=wt[f"f2b{l}"][:, :])
            for lw in range(NWL if STAGE >= 8 else 0):
                agT_ps = ppool.tile([P, P], f32, tag="ps")
                nc.tensor.transpose(out=agT_ps[:],
                                    in_=aggm[:, lw * P:(lw + 1) * P],
                                    identity=ident[:])
                agT = pool.tile([P, P], f32, tag="agT")
                nc.vector.tensor_copy(agT[:], agT_ps[:])
                o_ps = ppool.tile([P, D], f32, tag="ps")
                nc.tensor.matmul(out=o_ps[:], lhsT=agT[:], rhs=w_o[:],
                                 start=True, stop=False)
                nc.tensor.matmul(out=o_ps[:], lhsT=ones_row[:], rhs=b_o[:],
                                 start=False, stop=True)
                h_sb = pool.tile([P, D], f32, tag="hsb2")
                nc.sync.dma_start(out=h_sb[:],
                                  in_=A_src[lw * P:(lw + 1) * P, 0:128])
                h1 = pool.tile([P, D], f32, tag="h1")
                nc.vector.tensor_tensor(out=h1[:], in0=h_sb[:], in1=o_ps[:],
                                        op=Alu.add)
                hsqs = pool.tile([P, 1], f32, tag="hsqs2")
                mu, rstd = ln_stats(h1[:], hsqs[:], "l2")
                z2 = pool.tile([P, D], f32, tag="z2")
                nc.vector.tensor_scalar(out=z2[:], in0=h1[:], scalar1=mu[:],
                                        scalar2=rstd[:], op0=Alu.subtract,
                                        op1=Alu.mult)
                zT_ps = ppool.tile([P, P], f32, tag="ps")
                nc.tensor.transpose(out=zT_ps[:], in_=z2[:],
                                    identity=ident[:])
                zT = pool.tile([P, P], f32, tag="zT2")
                nc.vector.tensor_copy(zT[:], zT_ps[:])
                mid_ps = ppool.tile([P, 4 * D], f32, tag="ps")
                nc.tensor.matmul(out=mid_ps[:], lhsT=zT[:], rhs=w_f1[:],
                                 start=True, stop=False)
                nc.tensor.matmul(out=mid_ps[:], lhsT=ones_row[:],
                                 rhs=b_f1[:], start=False, stop=True)
                mid = pool.tile([P, 4 * D], f32, tag="mids")
                nc.vector.tensor_scalar(out=mid[:], in0=mid_ps[:],
                                        scalar1=0.0, scalar2=None,
                                        op0=Alu.max)
                f_ps = ppool.tile([P, D], f32, tag="ps")
                for c4 in range(4):
                    mT_ps = ppool.tile([P, P], f32, tag="ps")
                    nc.tensor.transpose(out=mT_ps[:],
                                        in_=mid[:, c4 * P:(c4 + 1) * P],
                                        identity=ident[:])
                    mT = pool.tile([P, P], f32, tag="mT")
                    nc.vector.tensor_copy(mT[:], mT_ps[:])
                    nc.tensor.matmul(out=f_ps[:], lhsT=mT[:],
                                     rhs=w_f2[:, c4 * D:(c4 + 1) * D],
                                     start=(c4 == 0), stop=False)
                nc.tensor.matmul(out=f_ps[:], lhsT=ones_row[:], rhs=b_f2[:],
                                 start=False, stop=True)
                h2 = pool.tile([P, D], f32, tag="h2")
                nc.vector.tensor_tensor(out=h2[:], in0=h1[:], in1=f_ps[:],
                                        op=Alu.add)
                nc.sync.dma_start(out=ag_in[l + 1][lw * P:(lw + 1) * P, :],
                                  in_=h2[:])
            nc.gpsimd.collective_compute(
                "AllGather", Alu.bypass, ins=[ag_in[l + 1][:, :]],
                outs=[h_full[l + 1][:, :]], replica_groups=RG)

        if DUMP:
            nc.sync.dma_start(out=dbg["dbg_h1"][:, :],
                              in_=h_full[min(1, L)][:, :])
        # ---- pooling + output MLP ----
        gs_ps = gpool.tile([G, D], f32, tag="gsum")
        NWp = NW if STAGE >= 9 else 1
        for w in range(NWp):
            h_sb = pool.tile([P, D], f32, tag="hsb")
            nc.sync.dma_start(out=h_sb[:],
                              in_=h_full[L][w * P:(w + 1) * P, :])
            Mg = pool.tile([P, G], f32, tag="Mg")
            nc.vector.tensor_tensor(
                out=Mg[:], in0=m_brel[:, w:w + 1].to_broadcast([P, G]),
                in1=iota64[:], op=Alu.is_equal)
            nc.tensor.matmul(out=gs_ps[:], lhsT=Mg[:], rhs=h_sb[:],
                             start=(w == 0), stop=(w == NWp - 1))
        gmean = pool.tile([G, D], f32, tag="gmean")
        nc.vector.tensor_scalar(out=gmean[:], in0=gs_ps[:], scalar1=ginv[:],
                                scalar2=None, op0=Alu.mult)
        w_o1 = wpool.tile([D, G], f32, tag="wo1")
        b_o1 = wpool.tile([1, G], f32, tag="bo1")
        w_o2 = wpool.tile([G, 1], f32, tag="wo2")
        b_o2 = wpool.tile([1, 1], f32, tag="bo2")
        nc.sync.dma_start(out=w_o1[:], in_=wt["outW1"][:, :])
        nc.sync.dma_start(out=b_o1[:], in_=wt["outb1"][:, :])
        nc.sync.dma_start(out=w_o2[:], in_=wt["outW2"][:, :])
        nc.sync.dma_start(out=b_o2[:], in_=wt["outb2"][:, :])
        gT_ps = ppool.tile([P, G], f32, tag="ps")
        nc.tensor.transpose(out=gT_ps[:, :], in_=gmean[:],
                            identity=ident[0:G, 0:G])
        gT = pool.tile([D, G], f32, tag="gTs")
        nc.vector.tensor_copy(gT[:], gT_ps[:])
        g1_ps = ppool.tile([G, G], f32, tag="ps")
        nc.tensor.matmul(out=g1_ps[:], lhsT=gT[:], rhs=w_o1[:],
                         start=True, stop=False)
        nc.tensor.matmul(out=g1_ps[:], lhsT=ones_row[:, 0:G], rhs=b_o1[:],
                         start=False, stop=True)
        g1 = pool.tile([G, G], f32, tag="g1s")
        nc.vector.tensor_scalar(out=g1[:], in0=g1_ps[:], scalar1=0.0,
                                scalar2=None, op0=Alu.max)
        g1T_ps = ppool.tile([G, G], f32, tag="ps")
        nc.tensor.transpose(out=g1T_ps[:], in_=g1[:],
                            identity=ident[0:G, 0:G])
        g1T = pool.tile([G, G], f32, tag="g1Ts")
        nc.vector.tensor_copy(g1T[:], g1T_ps[:])
        y_ps = ppool.tile([G, 1], f32, tag="ps")
        nc.tensor.matmul(out=y_ps[:], lhsT=g1T[:], rhs=w_o2[:],
                         start=True, stop=False)
        nc.tensor.matmul(out=y_ps[:], lhsT=ones_row[:, 0:G], rhs=b_o2[:],
                         start=False, stop=True)
        y_sb = pool.tile([G, 1], f32, tag="ysb")
        nc.vector.tensor_copy(y_sb[:], y_ps[:])
        nc.sync.dma_start(out=y_out[:, :], in_=y_sb[:])

    nc.compile()
    return nc


# ---------------------------------------------------------------------------
# runner
# ---------------------------------------------------------------------------

def _make_in_maps(inp):
    x = inp["x"].astype(np.float32)
    edge_index = inp["edge_index"].astype(np.int64)
    batch = inp["batch"].astype(np.int64)
    n_nodes = x.shape[0]

    meta, cores = _host_prep(x, edge_index)
    wf = _fold_weights(inp)

    counts = np.maximum(
        np.bincount(batch, minlength=G).astype(np.float32), 1.0)
    wmap = {"w_" + k: np.ascontiguousarray(v, np.float32)
            for k, v in wf.items()}
    wmap["ginv"] = (1.0 / counts).reshape(G, 1).astype(np.float32)

    in_maps = []
    for c in range(NC):
        m = _core_inputs(meta, cores[c], x, batch, n_nodes, c)
        m.update(wmap)
        in_maps.append(m)
    return meta, in_maps


def kernel(**inputs):
    global _LAST_EXEC_NS
    inp = {k: np.asarray(v) for k, v in inputs.items()}
    meta, in_maps = _make_in_maps(inp)

    from concourse.bass_utils import run_bass_kernel_spmd

    nc = _build_program(meta)
    res = run_bass_kernel_spmd(nc, in_maps, core_ids=list(range(NC)))
    t0 = time.time()
    res = run_bass_kernel_spmd(nc, in_maps, core_ids=list(range(NC)))
    _LAST_EXEC_NS = int((time.time() - t0) * 1e9)
    return np.asarray(res.results[0]["y"], np.float32)
